# revision 28
# baseline (speedup 1.0000x reference)
"""Trainium2 Bass kernel for nn_Conv2d_77489799955262.

Forward value of the reference:
    y = conv2d(x, (w_pos > 0) - (w_neg > 0))      # ternary weights in {-1, 0, 1}
(the straight-through-estimator terms cancel numerically), NCHW, 3x3, stride 1,
pad 1, x [32, 256, 56, 56] f32, w [256, 256, 3, 3].

Strategy: data-parallel over batch across 8 cores (4 images per core), then
1D Winograd F(2,3) along W inside each core:

    per output row pair (w=2j, 2j+1) the 3-tap conv along W becomes 4
    Winograd points; the 3 vertical taps stay as plain PSUM accumulation.
    v0 = d[2j]-d[2j+2], v1 = d[2j+1]+d[2j+2],
    v2 = d[2j+2]-d[2j+1], v3 = d[2j+1]-d[2j+3]       (DVE, from padded x)
    m_q = sum_{kh,ci} wt[q,kh]^T v_q[row+kh]          (PE, PSUM accumulate)
    y_even = m0+m1+m2,  y_odd = m1-m2-m3              (DVE)

This is 12 matmul columns per output element pair vs 18.3 for direct conv
(1.53x less PE work). Ternary weights make wt (multiples of 0.5) exact in
bf16; x rounded to bf16 inside the v-transform gives ~2e-3 rel err.

Layout: padded image planes with PW=57 (right pad of row r = left pad of
row r+1 share a zero), PH=58. v planes per (img, ci-block, q): [58 rows x
28 j] bf16. Matmul N = 14 out rows x 28 = 392 (fits one PSUM bank); 4
psum planes (q=0..3) per row block, double-buffered across the 8 banks.
"""
import numpy as np
import ml_dtypes

import concourse.bass as bass
import concourse.tile as tile
from concourse import bacc, mybir
from concourse.bass_utils import run_bass_kernel_spmd

MODE = "wino"           # kept for test.py compatibility
DUAL_PSUM = False       # walrus rejects tensor_tensor with 2 PSUM operands
# "carry": v(n+1)/x(n+2) pipelined across the For_i boundary
# "intra": same pipelining but self-contained per iteration (v0 burst at top)
# "off":   v(n) burst + x(n) DMA at each image start
PIPELINE = "intra"

N_CORES = 8
B, CI, CO, H, W, K = 32, 256, 256, 56, 56, 3
NI = B // N_CORES          # images per core
PH, PW = H + 2, W + 1      # 58 padded rows; row stride 57 (shared pad col)
IMG = PH * PW              # 3306 padded elems per image per channel
CIB = CI // 128            # ci blocks
COB = CO // 128            # co blocks
NJ = W // 2                # 28 winograd tiles per row
RB = 14                    # output rows per psum tile
NBLK = H // RB             # 4 row blocks
NMM = RB * NJ              # 392 matmul free dim
VROWS = PH                 # 58 v rows (0 and 57 are zero pads)

F32 = mybir.dt.float32
BF16 = mybir.dt.bfloat16

_COMPILED = {}

# (a_base, b_base, op) for v_q = d[a] op d[b] with col bases a,b in 0..3
_VDEF = [(0, 2, "sub"), (1, 2, "add"), (2, 1, "sub"), (1, 3, "sub")]


def _build(mode, iters=1, loop=0):
    nc = bacc.Bacc("TRN2", target_bir_lowering=False, debug=False,
                   num_devices=N_CORES)

    x_dram = nc.dram_tensor("x", [NI, CI, H, W], F32, kind="ExternalInput")
    w_dram = nc.dram_tensor("w", [CI, 4, 3, CO], BF16, kind="ExternalInput")
    y_dram = nc.dram_tensor("y", [NI, CO, H, W], F32, kind="ExternalOutput")

    with tile.TileContext(nc) as tc:
        with (
            tc.tile_pool(name="const", bufs=1) as cpool,
            tc.tile_pool(name="xst", bufs=2) as xpool,
            tc.tile_pool(name="vst", bufs=2) as vpool,
            tc.tile_pool(name="drain", bufs=4) as dpool,
            tc.tile_pool(name="outp", bufs=4) as opool,
            tc.tile_pool(name="psum", bufs=8, space="PSUM") as ppool,
        ):
            # weights: [128ci, cib, q, kh, co]; first-needed slice on the
            # sync queue ahead of x, the rest via SWDGE on gpsimd
            w_sb = cpool.tile([128, CIB, 4, 3, CO], BF16, tag="w")

            def w_slice(ci, co):
                return (w_sb[:, ci, :, :, co * 128:(co + 1) * 128],
                        w_dram[ci * 128:(ci + 1) * 128, :, :,
                               co * 128:(co + 1) * 128])

            nc.sync.dma_start(*w_slice(0, 0))
            for ci, co in ((1, 0), (0, 1), (1, 1)):
                nc.gpsimd.dma_start(*w_slice(ci, co))

            def stage_x_tile(it, n):
                return xpool.tile([128, CIB * H * W], F32, tag="x",
                                  name=f"x_{it}_{n}")

            def stage_x_chunk(xt, n, g):
                """DMA rows [7g, 7g+7) of both ci blocks of image n —
                emitted after each block's y DMA so the big x prefetch
                never head-of-line blocks the output queue."""
                r0, r1 = 7 * g, 7 * g + 7
                for ci in range(CIB):
                    nc.sync.dma_start(
                        xt[:, ci * H * W + r0 * W:ci * H * W + r1 * W],
                        x_dram[n, ci * 128:(ci + 1) * 128, r0:r1, :])

            def stage_x(it, n, first=False):
                xt = stage_x_tile(it, n)
                for g in range(8):
                    stage_x_chunk(xt, n, g)
                return xt

            def new_vt(it, n):
                vt = vpool.tile([128, CIB, 4, VROWS * NJ], BF16,
                                tag="v", name=f"v_{it}_{n}")
                # zero pad rows 0 and 57 of each v plane
                vv = vt[:].rearrange("p c q (r j) -> p (c q) r j", j=NJ)
                nc.gpsimd.memset(vv[:, :, 0:1, :], 0.0)
                nc.gpsimd.memset(vv[:, :, 57:58, :], 0.0)
                return vt

            def v_tt(xt, vt, j):
                """One v-transform plane (ci, q) = j//4, j%4, straight from
                the unpadded image; the single pad-affected edge column of
                v0/v3 is a cheap ACT op."""
                ci, q = divmod(j, 4)
                xv = (xt[:, ci * H * W:(ci + 1) * H * W]
                      .rearrange("p (r c) -> p r c", c=W))
                out = (vt[:, ci, q, NJ:NJ + 56 * NJ]
                       .rearrange("p (r j) -> p r j", j=NJ))
                if q == 0:
                    # v0 = x[2j-1] - x[2j+1]; j=0 edge: -x[:,1]
                    nc.vector.tensor_sub(out[:, :, 1:28],
                                         xv[:, :, 1:54:2],
                                         xv[:, :, 3:56:2])
                    nc.scalar.mul(out[:, :, 0:1], xv[:, :, 1:2], -1.0)
                elif q == 1:
                    nc.vector.tensor_add(out, xv[:, :, 0:56:2],
                                         xv[:, :, 1:56:2])
                elif q == 2:
                    nc.vector.tensor_sub(out, xv[:, :, 1:56:2],
                                         xv[:, :, 0:56:2])
                else:
                    # v3 = x[2j] - x[2j+2]; j=27 edge: +x[:,54]
                    nc.vector.tensor_sub(out[:, :, 0:27],
                                         xv[:, :, 0:54:2],
                                         xv[:, :, 2:56:2])
                    nc.scalar.copy(out[:, :, 27:28], xv[:, :, 54:55])

            # how many next-image v-transform ops to slot in after each of
            # the 8 (co, blk) groups: all 8 done by group 6 so the next
            # image's first matmul never waits
            _VSLOT = [1, 1, 1, 1, 1, 1, 2, 0]

            # pipeline state threaded across prologue + loop body:
            # st["xq"]: staged x tiles, oldest = image n+1 at image-n time
            # st["vt"]: v planes for the image about to be matmul'd
            st = {}

            def prologue():
                x0 = stage_x(0, 0, first=True)
                x1 = stage_x(0, 1)
                vt0 = new_vt(0, 0)
                for j in range(2 * 4):
                    v_tt(x0, vt0, j)
                st["xq"] = [x1]
                st["vt"] = vt0

            def emit_iter(it):
                if PIPELINE == "intra":
                    # self-contained per iteration: fill at body top, then
                    # v(n+1) interleaved with image n's drains; nothing in
                    # the body references tiles from outside the body
                    x0 = stage_x(it, 0)
                    x1 = stage_x(it, 1)
                    vt0 = new_vt(it, 0)
                    for j in range(2 * 4):
                        v_tt(x0, vt0, j)
                    xq, vt = [x1], vt0
                    for n in range(NI):
                        if n + 2 < NI:
                            xq.append(stage_x_tile(it, n + 2))
                            xt_pf = xq[-1]
                        else:
                            xt_pf = None
                        if n + 1 < NI:
                            vt_next = new_vt(it, n + 1)
                            xt_next = xq[0]
                        else:
                            vt_next = xt_next = None
                        emit_image(it, n, vt, xt_next, vt_next, xt_pf)
                        if n + 1 < NI:
                            xq.pop(0)
                            vt = vt_next
                    return
                for n in range(NI):
                    if PIPELINE == "off":
                        xt = stage_x(it, n)
                        vt = new_vt(it, n)
                        for j in range(2 * 4):
                            v_tt(xt, vt, j)
                        emit_image(it, n, vt, None, None)
                        continue
                    xt_pf = stage_x_tile(it, (n + 2) % NI)
                    st["xq"].append(xt_pf)
                    vt = st["vt"]
                    xt_next = st["xq"][0]
                    vt_next = new_vt(it, (n + 1) % NI)
                    emit_image(it, n, vt, xt_next, vt_next, xt_pf)
                    st["xq"].pop(0)
                    st["vt"] = vt_next

            def emit_image(it, n, vt, xt_next, vt_next, xt_pf=None):
                    vj = 0

                    for co in range(COB):
                        for blk in range(NBLK):
                            r0 = blk * RB
                            pss = [ppool.tile([128, NMM], F32, tag="ps",
                                              name=f"ps_{it}_{n}_{co}_{blk}_{q}")
                                   for q in range(4)]
                            for q in range(4):
                                for kh in range(3):
                                    for ci in range(CIB):
                                        lhsT = w_sb[:, ci, q, kh,
                                                    co * 128:(co + 1) * 128]
                                        rhs = vt[:, ci, q,
                                                 (r0 + kh) * NJ:
                                                 (r0 + kh + RB) * NJ]
                                        nc.tensor.matmul(
                                            pss[q][:], lhsT, rhs,
                                            start=(kh == 0 and ci == 0),
                                            stop=(kh == 2 and ci == 1))
                            # y_even = m0+m1+m2, y_odd = m1-m2-m3
                            ot = opool.tile([128, RB * W], F32, tag="ot",
                                            name=f"ot_{it}_{n}_{co}_{blk}")
                            ot_v = ot[:].rearrange("p (r c) -> p r c", c=W)
                            yE = ot_v[:, :, 0:56:2]
                            yO = ot_v[:, :, 1:56:2]
                            e1 = dpool.tile([128, NMM], F32, tag="e1",
                                            name=f"e1_{it}_{n}_{co}_{blk}")
                            o1 = dpool.tile([128, NMM], F32, tag="o1",
                                            name=f"o1_{it}_{n}_{co}_{blk}")
                            if DUAL_PSUM:
                                nc.vector.tensor_add(e1[:], pss[0][:],
                                                     pss[1][:])
                                nc.vector.tensor_sub(o1[:], pss[1][:],
                                                     pss[2][:])
                            else:
                                # ACT evacuates m1/m2 (fast PSUM port);
                                # DVE does the combines, each with at most
                                # one PSUM operand
                                c1 = dpool.tile([128, NMM], F32, tag="c1",
                                                name=f"c1_{it}_{n}_{co}_{blk}")
                                c2 = dpool.tile([128, NMM], F32, tag="c2",
                                                name=f"c2_{it}_{n}_{co}_{blk}")
                                nc.scalar.copy(c1[:], pss[1][:])
                                nc.scalar.copy(c2[:], pss[2][:])
                                nc.vector.tensor_add(e1[:], pss[0][:], c1[:])
                                nc.vector.tensor_sub(o1[:], c1[:], c2[:])
                            e1v = e1[:].rearrange("p (r j) -> p r j", j=NJ)
                            o1v = o1[:].rearrange("p (r j) -> p r j", j=NJ)
                            ps2v = (pss[2][:]
                                    .rearrange("p (r j) -> p r j", j=NJ))
                            ps3v = (pss[3][:]
                                    .rearrange("p (r j) -> p r j", j=NJ))
                            nc.vector.tensor_add(yE, e1v, ps2v)
                            nc.vector.tensor_sub(yO, o1v, ps3v)
                            if vt_next is not None:
                                for _ in range(_VSLOT[co * NBLK + blk]):
                                    v_tt(xt_next, vt_next, vj)
                                    vj += 1
                            nc.sync.dma_start(
                                y_dram[n, co * 128:(co + 1) * 128,
                                       r0:r0 + RB, :],
                                ot[:])
                            if xt_pf is not None:
                                stage_x_chunk(xt_pf, (n + 2) % NI,
                                              co * NBLK + blk)

            if PIPELINE == "carry":
                prologue()
            if loop:
                with tc.For_i(0, loop, 1,
                              hint_engines=(mybir.EngineType.PE,)):
                    emit_iter(0)
            else:
                for it in range(iters):
                    emit_iter(it)

    nc.compile()
    return nc


def _get_compiled(mode):
    if mode not in _COMPILED:
        _COMPILED[mode] = _build(mode)
    return _COMPILED[mode]


def _prep_weights(w_pos, w_neg, mode):
    w_eff = ((w_pos > 0).astype(np.float32)
             - (w_neg > 0).astype(np.float32))          # [CO, CI, 3, 3]
    w0, w1, w2 = w_eff[:, :, :, 0], w_eff[:, :, :, 1], w_eff[:, :, :, 2]
    g = np.stack([w0,
                  0.5 * (w0 + w1 + w2),
                  0.5 * (w0 - w1 + w2),
                  w2], axis=0)                          # [4q, CO, CI, 3kh]
    w_lhsT = np.ascontiguousarray(g.transpose(2, 0, 3, 1))  # [CI, q, kh, CO]
    return w_lhsT.astype(ml_dtypes.bfloat16)            # exact: k/2 values


def kernel(x, w_pos, w_neg):
    mode = MODE
    nc = _get_compiled(mode)
    w_lhsT = _prep_weights(w_pos, w_neg, mode)
    x = np.ascontiguousarray(x, dtype=np.float32)

    in_maps = [
        {"x": x[c * NI:(c + 1) * NI], "w": w_lhsT}
        for c in range(N_CORES)
    ]
    res = run_bass_kernel_spmd(nc, in_maps, list(range(N_CORES)))
    out = np.concatenate([res.results[c]["y"] for c in range(N_CORES)], axis=0)
    return out.astype(np.float32)


# revision 35
# speedup vs baseline: 1.0587x; 1.0587x over previous
"""Trainium2 Bass kernel for nn_Conv2d_77489799955262.

Forward value of the reference:
    y = conv2d(x, (w_pos > 0) - (w_neg > 0))      # ternary weights in {-1, 0, 1}
(the straight-through-estimator terms cancel numerically), NCHW, 3x3, stride 1,
pad 1, x [32, 256, 56, 56] f32, w [256, 256, 3, 3].

Strategy: data-parallel over batch across 8 cores (4 images per core), then
1D Winograd F(2,3) along W inside each core:

    per output row pair (w=2j, 2j+1) the 3-tap conv along W becomes 4
    Winograd points; the 3 vertical taps stay as plain PSUM accumulation.
    v0 = d[2j]-d[2j+2], v1 = d[2j+1]+d[2j+2],
    v2 = d[2j+2]-d[2j+1], v3 = d[2j+1]-d[2j+3]       (DVE, from padded x)
    m_q = sum_{kh,ci} wt[q,kh]^T v_q[row+kh]          (PE, PSUM accumulate)
    y_even = m0+m1+m2,  y_odd = m1-m2-m3              (DVE)

This is 12 matmul columns per output element pair vs 18.3 for direct conv
(1.53x less PE work). Ternary weights make wt (multiples of 0.5) exact in
bf16; x rounded to bf16 inside the v-transform gives ~2e-3 rel err.

Layout: padded image planes with PW=57 (right pad of row r = left pad of
row r+1 share a zero), PH=58. v planes per (img, ci-block, q): [58 rows x
28 j] bf16. Matmul N = 14 out rows x 28 = 392 (fits one PSUM bank); 4
psum planes (q=0..3) per row block, double-buffered across the 8 banks.
"""
import numpy as np
import ml_dtypes

import concourse.bass as bass
import concourse.tile as tile
from concourse import bacc, mybir
from concourse.bass_utils import run_bass_kernel_spmd

MODE = "wino"           # kept for test.py compatibility
DUAL_PSUM = False       # walrus rejects tensor_tensor with 2 PSUM operands
# "carry": v(n+1)/x(n+2) pipelined across the For_i boundary
# "intra": same pipelining but self-contained per iteration (v0 burst at top)
# "off":   v(n) burst + x(n) DMA at each image start
PIPELINE = "carry"

N_CORES = 8
B, CI, CO, H, W, K = 32, 256, 256, 56, 56, 3
NI = B // N_CORES          # images per core
PH, PW = H + 2, W + 1      # 58 padded rows; row stride 57 (shared pad col)
IMG = PH * PW              # 3306 padded elems per image per channel
CIB = CI // 128            # ci blocks
COB = CO // 128            # co blocks
NJ = W // 2                # 28 winograd tiles per row
RB = 14                    # output rows per psum tile
NBLK = H // RB             # 4 row blocks
NMM = RB * NJ              # 392 matmul free dim
VROWS = PH                 # 58 v rows (0 and 57 are zero pads)

F32 = mybir.dt.float32
BF16 = mybir.dt.bfloat16

_COMPILED = {}

# (a_base, b_base, op) for v_q = d[a] op d[b] with col bases a,b in 0..3
_VDEF = [(0, 2, "sub"), (1, 2, "add"), (2, 1, "sub"), (1, 3, "sub")]


def _build(mode, iters=1, loop=0):
    nc = bacc.Bacc("TRN2", target_bir_lowering=False, debug=False,
                   num_devices=N_CORES)

    x_dram = nc.dram_tensor("x", [NI, CI, H, W], F32, kind="ExternalInput")
    w_dram = nc.dram_tensor("w", [CI, 4, 3, CO], BF16, kind="ExternalInput")
    y_dram = nc.dram_tensor("y", [NI, CO, H, W], F32, kind="ExternalOutput")

    with tile.TileContext(nc) as tc:
        with (
            tc.tile_pool(name="const", bufs=1) as cpool,
            tc.tile_pool(name="xst", bufs=2) as xpool,
            tc.tile_pool(name="vst", bufs=2) as vpool,
            tc.tile_pool(name="drain", bufs=4) as dpool,
            tc.tile_pool(name="outp", bufs=4) as opool,
            tc.tile_pool(name="psum", bufs=8, space="PSUM") as ppool,
        ):
            # weights: [128ci, cib, q, kh, co]; first-needed slice on the
            # sync queue ahead of x, the rest via SWDGE on gpsimd
            w_sb = cpool.tile([128, CIB, 4, 3, CO], BF16, tag="w")

            def w_slice(ci, co):
                return (w_sb[:, ci, :, :, co * 128:(co + 1) * 128],
                        w_dram[ci * 128:(ci + 1) * 128, :, :,
                               co * 128:(co + 1) * 128])

            nc.sync.dma_start(*w_slice(0, 0))
            for ci, co in ((1, 0), (0, 1), (1, 1)):
                nc.gpsimd.dma_start(*w_slice(ci, co))

            def stage_x(it, n, first=False):
                """Padded-plane staging: zero the pad borders (shared-pad
                PW=57 layout), then one strided interior DMA per ci block."""
                xt = xpool.tile([128, CIB * IMG], F32, tag="x",
                                name=f"x_{it}_{n}")
                for ci in range(CIB):
                    o = ci * IMG
                    eng = nc.vector if first else nc.gpsimd
                    eng.memset(xt[:, o:o + PW + 1], 0.0)
                    cols = (xt[:, o + 2 * PW:o + 2 * PW + 55 * PW]
                            .rearrange("p (r c) -> p r c", c=PW)
                            [:, :, 0:1])
                    eng.memset(cols, 0.0)
                    eng.memset(xt[:, o + 57 * PW:o + IMG], 0.0)
                    interior = (xt[:, o:o + IMG]
                                .rearrange("p (r c) -> p r c", c=PW)
                                [:, 1:1 + H, 1:1 + W])
                    nc.sync.dma_start(
                        interior, x_dram[n, ci * 128:(ci + 1) * 128, :, :])
                return xt

            def new_vt(it, n):
                vt = vpool.tile([128, CIB, 4, VROWS * NJ], BF16,
                                tag="v", name=f"v_{it}_{n}")
                # zero pad rows 0 and 57 of each v plane
                vv = vt[:].rearrange("p c q (r j) -> p (c q) r j", j=NJ)
                nc.gpsimd.memset(vv[:, :, 0:1, :], 0.0)
                nc.gpsimd.memset(vv[:, :, 57:58, :], 0.0)
                return vt

            def v_tt(xt, vt, j):
                """One v-transform op: plane (ci, q) = j//4, j%4, from the
                padded plane (pads supply the boundary zeros)."""
                ci, q = divmod(j, 4)
                a0, b0, op = _VDEF[q]
                plane = xt[:, ci * IMG:(ci + 1) * IMG]

                def dview(c0):
                    return (plane[:, c0 + PW:c0 + PW + 56 * PW]
                            .rearrange("p (r c) -> p r c", c=PW)
                            [:, :, 0:56:2])
                out = (vt[:, ci, q, NJ:NJ + 56 * NJ]
                       .rearrange("p (r j) -> p r j", j=NJ))
                f = (nc.vector.tensor_add if op == "add"
                     else nc.vector.tensor_sub)
                f(out, dview(a0), dview(b0))

            # how many next-image v-transform ops to slot in after each of
            # the 8 (co, blk) groups: all 8 done by group 6 so the next
            # image's first matmul never waits
            _VSLOT = [1, 1, 1, 1, 1, 1, 2, 0]

            # pipeline state threaded across prologue + loop body:
            # st["xq"]: staged x tiles, oldest = image n+1 at image-n time
            # st["vt"]: v planes for the image about to be matmul'd
            st = {}

            def prologue():
                x0 = stage_x(0, 0, first=True)
                x1 = stage_x(0, 1)
                vt0 = new_vt(0, 0)
                for j in range(2 * 4):
                    v_tt(x0, vt0, j)
                st["xq"] = [x1]
                st["vt"] = vt0

            def emit_iter(it):
                if PIPELINE == "intra":
                    # self-contained per iteration: fill at body top, then
                    # v(n+1) interleaved with image n's drains; nothing in
                    # the body references tiles from outside the body
                    x0 = stage_x(it, 0)
                    x1 = stage_x(it, 1)
                    vt0 = new_vt(it, 0)
                    for j in range(2 * 4):
                        v_tt(x0, vt0, j)
                    xq, vt = [x1], vt0
                    for n in range(NI):
                        if n + 2 < NI:
                            xq.append(stage_x(it, n + 2))
                        if n + 1 < NI:
                            vt_next = new_vt(it, n + 1)
                            xt_next = xq[0]
                        else:
                            vt_next = xt_next = None
                        emit_image(it, n, vt, xt_next, vt_next)
                        if n + 1 < NI:
                            xq.pop(0)
                            vt = vt_next
                    return
                for n in range(NI):
                    if PIPELINE == "off":
                        xt = stage_x(it, n)
                        vt = new_vt(it, n)
                        for j in range(2 * 4):
                            v_tt(xt, vt, j)
                        emit_image(it, n, vt, None, None)
                        continue
                    st["xq"].append(stage_x(it, (n + 2) % NI))
                    vt = st["vt"]
                    xt_next = st["xq"][0]
                    vt_next = new_vt(it, (n + 1) % NI)
                    emit_image(it, n, vt, xt_next, vt_next)
                    st["xq"].pop(0)
                    st["vt"] = vt_next

            def emit_image(it, n, vt, xt_next, vt_next):
                    vj = 0

                    for co in range(COB):
                        for blk in range(NBLK):
                            r0 = blk * RB
                            pss = [ppool.tile([128, NMM], F32, tag="ps",
                                              name=f"ps_{it}_{n}_{co}_{blk}_{q}")
                                   for q in range(4)]
                            for q in range(4):
                                for kh in range(3):
                                    for ci in range(CIB):
                                        lhsT = w_sb[:, ci, q, kh,
                                                    co * 128:(co + 1) * 128]
                                        rhs = vt[:, ci, q,
                                                 (r0 + kh) * NJ:
                                                 (r0 + kh + RB) * NJ]
                                        nc.tensor.matmul(
                                            pss[q][:], lhsT, rhs,
                                            start=(kh == 0 and ci == 0),
                                            stop=(kh == 2 and ci == 1))
                            # y_even = m0+m1+m2, y_odd = m1-m2-m3
                            ot = opool.tile([128, RB * W], F32, tag="ot",
                                            name=f"ot_{it}_{n}_{co}_{blk}")
                            ot_v = ot[:].rearrange("p (r c) -> p r c", c=W)
                            yE = ot_v[:, :, 0:56:2]
                            yO = ot_v[:, :, 1:56:2]
                            e1 = dpool.tile([128, NMM], F32, tag="e1",
                                            name=f"e1_{it}_{n}_{co}_{blk}")
                            o1 = dpool.tile([128, NMM], F32, tag="o1",
                                            name=f"o1_{it}_{n}_{co}_{blk}")
                            if DUAL_PSUM:
                                nc.vector.tensor_add(e1[:], pss[0][:],
                                                     pss[1][:])
                                nc.vector.tensor_sub(o1[:], pss[1][:],
                                                     pss[2][:])
                            else:
                                # ACT evacuates m1/m2 (fast PSUM port);
                                # DVE does the combines, each with at most
                                # one PSUM operand
                                c1 = dpool.tile([128, NMM], F32, tag="c1",
                                                name=f"c1_{it}_{n}_{co}_{blk}")
                                c2 = dpool.tile([128, NMM], F32, tag="c2",
                                                name=f"c2_{it}_{n}_{co}_{blk}")
                                nc.scalar.copy(c1[:], pss[1][:])
                                nc.scalar.copy(c2[:], pss[2][:])
                                nc.vector.tensor_add(e1[:], pss[0][:], c1[:])
                                nc.vector.tensor_sub(o1[:], c1[:], c2[:])
                            e1v = e1[:].rearrange("p (r j) -> p r j", j=NJ)
                            o1v = o1[:].rearrange("p (r j) -> p r j", j=NJ)
                            ps2v = (pss[2][:]
                                    .rearrange("p (r j) -> p r j", j=NJ))
                            ps3v = (pss[3][:]
                                    .rearrange("p (r j) -> p r j", j=NJ))
                            nc.vector.tensor_add(yE, e1v, ps2v)
                            nc.vector.tensor_sub(yO, o1v, ps3v)
                            if vt_next is not None:
                                for _ in range(_VSLOT[co * NBLK + blk]):
                                    v_tt(xt_next, vt_next, vj)
                                    vj += 1
                            nc.sync.dma_start(
                                y_dram[n, co * 128:(co + 1) * 128,
                                       r0:r0 + RB, :],
                                ot[:])

            if PIPELINE == "carry":
                prologue()
            if loop:
                with tc.For_i(0, loop, 1,
                              hint_engines=(mybir.EngineType.PE,)):
                    emit_iter(0)
            else:
                for it in range(iters):
                    emit_iter(it)

    nc.compile()
    return nc


def _get_compiled(mode):
    if mode not in _COMPILED:
        _COMPILED[mode] = _build(mode)
    return _COMPILED[mode]


def _prep_weights(w_pos, w_neg, mode):
    w_eff = ((w_pos > 0).astype(np.float32)
             - (w_neg > 0).astype(np.float32))          # [CO, CI, 3, 3]
    w0, w1, w2 = w_eff[:, :, :, 0], w_eff[:, :, :, 1], w_eff[:, :, :, 2]
    g = np.stack([w0,
                  0.5 * (w0 + w1 + w2),
                  0.5 * (w0 - w1 + w2),
                  w2], axis=0)                          # [4q, CO, CI, 3kh]
    w_lhsT = np.ascontiguousarray(g.transpose(2, 0, 3, 1))  # [CI, q, kh, CO]
    return w_lhsT.astype(ml_dtypes.bfloat16)            # exact: k/2 values


def kernel(x, w_pos, w_neg):
    mode = MODE
    nc = _get_compiled(mode)
    w_lhsT = _prep_weights(w_pos, w_neg, mode)
    x = np.ascontiguousarray(x, dtype=np.float32)

    in_maps = [
        {"x": x[c * NI:(c + 1) * NI], "w": w_lhsT}
        for c in range(N_CORES)
    ]
    res = run_bass_kernel_spmd(nc, in_maps, list(range(N_CORES)))
    out = np.concatenate([res.results[c]["y"] for c in range(N_CORES)], axis=0)
    return out.astype(np.float32)


# revision 38
# speedup vs baseline: 1.2026x; 1.1359x over previous
"""Trainium2 Bass kernel for nn_Conv2d_77489799955262.

Forward value of the reference:
    y = conv2d(x, (w_pos > 0) - (w_neg > 0))      # ternary weights in {-1, 0, 1}
(the straight-through-estimator terms cancel numerically), NCHW, 3x3, stride 1,
pad 1, x [32, 256, 56, 56] f32, w [256, 256, 3, 3].

Strategy: data-parallel over batch across 8 cores (4 images per core), then
1D Winograd F(2,3) along W inside each core:

    per output row pair (w=2j, 2j+1) the 3-tap conv along W becomes 4
    Winograd points; the 3 vertical taps stay as plain PSUM accumulation.
    v0 = d[2j]-d[2j+2], v1 = d[2j+1]+d[2j+2],
    v2 = d[2j+2]-d[2j+1], v3 = d[2j+1]-d[2j+3]       (DVE, from padded x)
    m_q = sum_{kh,ci} wt[q,kh]^T v_q[row+kh]          (PE, PSUM accumulate)
    y_even = m0+m1+m2,  y_odd = m1-m2-m3              (DVE)

This is 12 matmul columns per output element pair vs 18.3 for direct conv
(1.53x less PE work). Ternary weights make wt (multiples of 0.5) exact in
bf16; x rounded to bf16 inside the v-transform gives ~2e-3 rel err.

Layout: padded image planes with PW=57 (right pad of row r = left pad of
row r+1 share a zero), PH=58. v planes per (img, ci-block, q): [58 rows x
28 j] bf16. Matmul N = 14 out rows x 28 = 392 (fits one PSUM bank); 4
psum planes (q=0..3) per row block, double-buffered across the 8 banks.
"""
import numpy as np
import ml_dtypes

import concourse.bass as bass
import concourse.tile as tile
from concourse import bacc, mybir
from concourse.bass_utils import run_bass_kernel_spmd

MODE = "wino"           # kept for test.py compatibility
DUAL_PSUM = False       # walrus rejects tensor_tensor with 2 PSUM operands
# "carry": v(n+1)/x(n+2) pipelined across the For_i boundary
# "intra": same pipelining but self-contained per iteration (v0 burst at top)
# "off":   v(n) burst + x(n) DMA at each image start
PIPELINE = "intra"

N_CORES = 8
B, CI, CO, H, W, K = 32, 256, 256, 56, 56, 3
NI = B // N_CORES          # images per core
PH, PW = H + 2, W + 1      # 58 padded rows; row stride 57 (shared pad col)
IMG = PH * PW              # 3306 padded elems per image per channel
CIB = CI // 128            # ci blocks
COB = CO // 128            # co blocks
NJ = W // 2                # 28 winograd tiles per row
RB = 14                    # output rows per psum tile
NBLK = H // RB             # 4 row blocks
NMM = RB * NJ              # 392 matmul free dim
VROWS = PH                 # 58 v rows (0 and 57 are zero pads)

F32 = mybir.dt.float32
BF16 = mybir.dt.bfloat16

_COMPILED = {}

# (a_base, b_base, op) for v_q = d[a] op d[b] with col bases a,b in 0..3
_VDEF = [(0, 2, "sub"), (1, 2, "add"), (2, 1, "sub"), (1, 3, "sub")]


def _build(mode, iters=1, loop=0):
    nc = bacc.Bacc("TRN2", target_bir_lowering=False, debug=False,
                   num_devices=N_CORES)

    x_dram = nc.dram_tensor("x", [NI, CI, H, W], F32, kind="ExternalInput")
    w_dram = nc.dram_tensor("w", [CI, 4, 3, CO], BF16, kind="ExternalInput")
    y_dram = nc.dram_tensor("y", [NI, CO, H, W], F32, kind="ExternalOutput")

    with tile.TileContext(nc) as tc:
        with (
            tc.tile_pool(name="const", bufs=1) as cpool,
            tc.tile_pool(name="xst", bufs=2) as xpool,
            tc.tile_pool(name="vst", bufs=2) as vpool,
            tc.tile_pool(name="drain", bufs=4) as dpool,
            tc.tile_pool(name="outp", bufs=4) as opool,
            tc.tile_pool(name="psum", bufs=8, space="PSUM") as ppool,
        ):
            # weights: [128ci, cib, q, kh, co]; first-needed slice on the
            # sync queue ahead of x, the rest via SWDGE on gpsimd
            w_sb = cpool.tile([128, CIB, 4, 3, CO], BF16, tag="w")

            def w_slice(ci, co):
                return (w_sb[:, ci, :, :, co * 128:(co + 1) * 128],
                        w_dram[ci * 128:(ci + 1) * 128, :, :,
                               co * 128:(co + 1) * 128])

            nc.sync.dma_start(*w_slice(0, 0))
            for ci, co in ((1, 0), (0, 1), (1, 1)):
                nc.gpsimd.dma_start(*w_slice(ci, co))

            def stage_x(it, n, first=False):
                """Padded-plane staging: zero the pad borders (shared-pad
                PW=57 layout), then one strided interior DMA per ci block."""
                xt = xpool.tile([128, CIB * IMG], F32, tag="x",
                                name=f"x_{it}_{n}")
                for ci in range(CIB):
                    o = ci * IMG
                    eng = nc.vector if first else nc.gpsimd
                    eng.memset(xt[:, o:o + PW + 1], 0.0)
                    cols = (xt[:, o + 2 * PW:o + 2 * PW + 55 * PW]
                            .rearrange("p (r c) -> p r c", c=PW)
                            [:, :, 0:1])
                    eng.memset(cols, 0.0)
                    eng.memset(xt[:, o + 57 * PW:o + IMG], 0.0)
                    interior = (xt[:, o:o + IMG]
                                .rearrange("p (r c) -> p r c", c=PW)
                                [:, 1:1 + H, 1:1 + W])
                    nc.sync.dma_start(
                        interior, x_dram[n, ci * 128:(ci + 1) * 128, :, :])
                return xt

            def new_vt(it, n):
                vt = vpool.tile([128, CIB, 4, VROWS * NJ], BF16,
                                tag="v", name=f"v_{it}_{n}")
                # zero pad rows 0 and 57 of each v plane
                vv = vt[:].rearrange("p c q (r j) -> p (c q) r j", j=NJ)
                nc.gpsimd.memset(vv[:, :, 0:1, :], 0.0)
                nc.gpsimd.memset(vv[:, :, 57:58, :], 0.0)
                return vt

            def v_tt(xt, vt, j, eng=None):
                """One v-transform op: plane (ci, q) = j//4, j%4, from the
                padded plane (pads supply the boundary zeros)."""
                eng = eng or nc.vector
                ci, q = divmod(j, 4)
                a0, b0, op = _VDEF[q]
                plane = xt[:, ci * IMG:(ci + 1) * IMG]

                def dview(c0):
                    return (plane[:, c0 + PW:c0 + PW + 56 * PW]
                            .rearrange("p (r c) -> p r c", c=PW)
                            [:, :, 0:56:2])
                out = (vt[:, ci, q, NJ:NJ + 56 * NJ]
                       .rearrange("p (r j) -> p r j", j=NJ))
                f = eng.tensor_add if op == "add" else eng.tensor_sub
                f(out, dview(a0), dview(b0))

            # off-mode burst: interleave DVE (5 planes) and GPSIMD (3
            # planes, the later-consumed ones) so the pre-matmul window is
            # max(8.3, 9.9) instead of 13.2 us; ordered so PE's q-major
            # consumption is fed earliest-first
            _VBURST = [(0, None), (6, "g"), (4, None), (3, "g"),
                       (1, None), (7, "g"), (5, None), (2, None)]

            # how many next-image v-transform ops to slot in after each of
            # the 8 (co, blk) groups: all 8 done by group 6 so the next
            # image's first matmul never waits
            _VSLOT = [1, 1, 1, 1, 1, 1, 2, 0]

            # pipeline state threaded across prologue + loop body:
            # st["xq"]: staged x tiles, oldest = image n+1 at image-n time
            # st["vt"]: v planes for the image about to be matmul'd
            st = {}

            def prologue():
                x0 = stage_x(0, 0, first=True)
                x1 = stage_x(0, 1)
                vt0 = new_vt(0, 0)
                for j in range(2 * 4):
                    v_tt(x0, vt0, j)
                st["xq"] = [x1]
                st["vt"] = vt0

            def emit_iter(it):
                if PIPELINE == "intra":
                    # self-contained per iteration: fill at body top, then
                    # v(n+1) interleaved with image n's drains; nothing in
                    # the body references tiles from outside the body
                    x0 = stage_x(it, 0)
                    x1 = stage_x(it, 1)
                    vt0 = new_vt(it, 0)
                    for j in range(2 * 4):
                        v_tt(x0, vt0, j)
                    xq, vt = [x1], vt0
                    for n in range(NI):
                        if n + 2 < NI:
                            xq.append(stage_x(it, n + 2))
                        if n + 1 < NI:
                            vt_next = new_vt(it, n + 1)
                            xt_next = xq[0]
                        else:
                            vt_next = xt_next = None
                        emit_image(it, n, vt, xt_next, vt_next)
                        if n + 1 < NI:
                            xq.pop(0)
                            vt = vt_next
                    return
                for n in range(NI):
                    if PIPELINE == "off":
                        xt = stage_x(it, n)
                        vt = new_vt(it, n)
                        for j, g in _VBURST:
                            v_tt(xt, vt, j, nc.gpsimd if g else None)
                        emit_image(it, n, vt, None, None)
                        continue
                    st["xq"].append(stage_x(it, (n + 2) % NI))
                    vt = st["vt"]
                    xt_next = st["xq"][0]
                    vt_next = new_vt(it, (n + 1) % NI)
                    emit_image(it, n, vt, xt_next, vt_next)
                    st["xq"].pop(0)
                    st["vt"] = vt_next

            def emit_image(it, n, vt, xt_next, vt_next):
                    vj = 0

                    for co in range(COB):
                        for blk in range(NBLK):
                            r0 = blk * RB
                            pss = [ppool.tile([128, NMM], F32, tag="ps",
                                              name=f"ps_{it}_{n}_{co}_{blk}_{q}")
                                   for q in range(4)]
                            for q in range(4):
                                for kh in range(3):
                                    for ci in range(CIB):
                                        lhsT = w_sb[:, ci, q, kh,
                                                    co * 128:(co + 1) * 128]
                                        rhs = vt[:, ci, q,
                                                 (r0 + kh) * NJ:
                                                 (r0 + kh + RB) * NJ]
                                        nc.tensor.matmul(
                                            pss[q][:], lhsT, rhs,
                                            start=(kh == 0 and ci == 0),
                                            stop=(kh == 2 and ci == 1))
                            # y_even = m0+m1+m2, y_odd = m1-m2-m3
                            ot = opool.tile([128, RB * W], F32, tag="ot",
                                            name=f"ot_{it}_{n}_{co}_{blk}")
                            ot_v = ot[:].rearrange("p (r c) -> p r c", c=W)
                            yE = ot_v[:, :, 0:56:2]
                            yO = ot_v[:, :, 1:56:2]
                            e1 = dpool.tile([128, NMM], F32, tag="e1",
                                            name=f"e1_{it}_{n}_{co}_{blk}")
                            o1 = dpool.tile([128, NMM], F32, tag="o1",
                                            name=f"o1_{it}_{n}_{co}_{blk}")
                            if DUAL_PSUM:
                                nc.vector.tensor_add(e1[:], pss[0][:],
                                                     pss[1][:])
                                nc.vector.tensor_sub(o1[:], pss[1][:],
                                                     pss[2][:])
                            else:
                                # ACT evacuates m1/m2 (fast PSUM port);
                                # DVE does the combines, each with at most
                                # one PSUM operand
                                c1 = dpool.tile([128, NMM], F32, tag="c1",
                                                name=f"c1_{it}_{n}_{co}_{blk}")
                                c2 = dpool.tile([128, NMM], F32, tag="c2",
                                                name=f"c2_{it}_{n}_{co}_{blk}")
                                nc.scalar.copy(c1[:], pss[1][:])
                                nc.scalar.copy(c2[:], pss[2][:])
                                nc.vector.tensor_add(e1[:], pss[0][:], c1[:])
                                nc.vector.tensor_sub(o1[:], c1[:], c2[:])
                            e1v = e1[:].rearrange("p (r j) -> p r j", j=NJ)
                            o1v = o1[:].rearrange("p (r j) -> p r j", j=NJ)
                            ps2v = (pss[2][:]
                                    .rearrange("p (r j) -> p r j", j=NJ))
                            ps3v = (pss[3][:]
                                    .rearrange("p (r j) -> p r j", j=NJ))
                            nc.vector.tensor_add(yE, e1v, ps2v)
                            nc.vector.tensor_sub(yO, o1v, ps3v)
                            if vt_next is not None:
                                for _ in range(_VSLOT[co * NBLK + blk]):
                                    v_tt(xt_next, vt_next, vj)
                                    vj += 1
                            nc.sync.dma_start(
                                y_dram[n, co * 128:(co + 1) * 128,
                                       r0:r0 + RB, :],
                                ot[:])

            if PIPELINE == "carry":
                prologue()
            if loop:
                with tc.For_i(0, loop, 1,
                              hint_engines=(mybir.EngineType.PE,)):
                    emit_iter(0)
            else:
                for it in range(iters):
                    emit_iter(it)

    nc.compile()
    return nc


def _get_compiled(mode):
    if mode not in _COMPILED:
        _COMPILED[mode] = _build(mode)
    return _COMPILED[mode]


def _prep_weights(w_pos, w_neg, mode):
    w_eff = ((w_pos > 0).astype(np.float32)
             - (w_neg > 0).astype(np.float32))          # [CO, CI, 3, 3]
    w0, w1, w2 = w_eff[:, :, :, 0], w_eff[:, :, :, 1], w_eff[:, :, :, 2]
    g = np.stack([w0,
                  0.5 * (w0 + w1 + w2),
                  0.5 * (w0 - w1 + w2),
                  w2], axis=0)                          # [4q, CO, CI, 3kh]
    w_lhsT = np.ascontiguousarray(g.transpose(2, 0, 3, 1))  # [CI, q, kh, CO]
    return w_lhsT.astype(ml_dtypes.bfloat16)            # exact: k/2 values


def kernel(x, w_pos, w_neg):
    mode = MODE
    nc = _get_compiled(mode)
    w_lhsT = _prep_weights(w_pos, w_neg, mode)
    x = np.ascontiguousarray(x, dtype=np.float32)

    in_maps = [
        {"x": x[c * NI:(c + 1) * NI], "w": w_lhsT}
        for c in range(N_CORES)
    ]
    res = run_bass_kernel_spmd(nc, in_maps, list(range(N_CORES)))
    out = np.concatenate([res.results[c]["y"] for c in range(N_CORES)], axis=0)
    return out.astype(np.float32)
